# revision 1
# baseline (speedup 1.0000x reference)
"""Batched CBF-QP safety filter on 8 Trainium2 NeuronCores.

Strategy (pure data parallel over the batch, per the sharding hint):
  - Shard batch 16384 -> 8 cores x 2048 samples. One SPMD NEFF, 8 in_maps.
  - Per core, 16 tiles of 128 samples (sample-major: partition = sample).
  - PE computes gh = -(Qc x + cc) for all (m,i) via one shared-weight matmul
    (host-prepacked [65, 520] constant: includes the affine part of h too).
  - DVE computes the per-sample contractions (Ax, W = gh@B, dots, P = 0.5WW^T+0.05I)
    as broadcast-product + segmented-reduce pairs.
  - The 16-dim dual QP of the reference provably reduces to an 8-dim NNQP
    min_{lam>=0} 0.5 lam^T P lam - q^T lam  (the s-block multipliers are 0 at
    the optimum, and FISTA-250 of the reference is converged to ~1e-4 of that
    optimum).  Solved exactly with 5 primal-dual active set (Hintermueller)
    iterations, each an 8x8 masked LDL^T solve done in SIMD across samples.
  - Recovery a = a_des + 0.5 W^T lam, DMA out.
"""

import os
import time
from contextlib import ExitStack

import numpy as np

import concourse.bacc as bacc
import concourse.bass as bass
import concourse.mybir as mybir
import concourse.tile as tile
from concourse.tile_rust import add_dep_helper

F32 = mybir.dt.float32
OP = mybir.AluOpType
AX = mybir.AxisListType

BATCH = 16384
XD = 64
AD = 16
NC = 8
PEN = 10.0
DELTA = 1.0
NCORES = 8
P128 = 128
PDAS_ITERS = 5

_last_result = None  # BassKernelResults of the most recent hardware run
_exec_wall = [None]  # wall seconds of the most recent run_bass_kernel_spmd call


def _qker_const(Qc: np.ndarray, cc: np.ndarray, dc: np.ndarray) -> np.ndarray:
    """[65, 520] fp32: gh columns (512) + affine-h columns (8).

    gh[s, 64*m+i]   = sum_j x[s,j] * (-Qc[m,i,j])  +  1 * (-cc[m,i])
    haff[s, m]      = sum_j x[s,j] * (-0.5*cc[m,j]) + 1 * dc[m]
    where the matmul lhsT is xaT = [x | 1]^T  ([65, 128] per tile).
    """
    k = np.zeros((65, 520), np.float32)
    # (m, i) major columns
    k[:64, :512] = -np.transpose(Qc, (2, 0, 1)).reshape(64, 512)
    k[64, :512] = -cc.reshape(512)
    k[:64, 512:520] = -0.5 * cc.T
    k[64, 512:520] = dc
    return k


def _ap(base: bass.AP, off_elems: int, dims):
    """Custom free-dim view of an SBUF/PSUM tile AP (keeps partition dim)."""
    return bass.AP(
        tensor=base.tensor,
        offset=base.offset + off_elems,
        ap=[list(base.ap[0])] + [list(d) for d in dims],
    )


def build_program(S: int, gpsimd_offload: bool = True, debug: bool = False):
    """Build the per-core Bass program for S samples (S % 128 == 0)."""
    T = S // P128
    nc = bacc.Bacc("TRN2", target_bir_lowering=False)

    d_x = nc.dram_tensor("x", [S, XD], F32, kind="ExternalInput").ap()
    d_xT = nc.dram_tensor("xT", [65, S], F32, kind="ExternalInput").ap()
    d_ades = nc.dram_tensor("a_des", [S, AD], F32, kind="ExternalInput").ap()
    d_A = nc.dram_tensor("A", [S, XD * XD], F32, kind="ExternalInput").ap()
    d_B = nc.dram_tensor("B", [S, XD * AD], F32, kind="ExternalInput").ap()
    d_qk = nc.dram_tensor("qker", [65, 520], F32, kind="ExternalInput").ap()
    d_out = nc.dram_tensor("a_safe", [S, AD], F32, kind="ExternalOutput").ap()
    dbg = {}
    if debug:
        for nm, sh in [("gh", [P128, 512]), ("Ax", [P128, 64]),
                       ("W", [P128, T * 128]), ("Pmat", [P128, T * 64]),
                       ("qv", [P128, T * 8]), ("lamv", [P128, T * 8]),
                       ("haffv", [P128, 8])]:
            dbg[nm] = nc.dram_tensor("dbg_" + nm, sh, F32, kind="ExternalOutput").ap()

    with tile.TileContext(nc) as tc, ExitStack() as ctx:
        consts = ctx.enter_context(tc.tile_pool(name="consts", bufs=1))
        dpool = ctx.enter_context(tc.tile_pool(name="dma", bufs=2))
        work = ctx.enter_context(tc.tile_pool(name="work", bufs=1))
        small = ctx.enter_context(tc.tile_pool(name="small", bufs=2))
        psum = ctx.enter_context(tc.tile_pool(name="psum", bufs=2, space="PSUM"))
        psum1 = ctx.enter_context(tc.tile_pool(name="psum1", bufs=2, space="PSUM"))

        # --- constants ---
        qker = consts.tile([65, 520], F32)
        nc.sync.dma_start(out=qker, in_=d_qk)
        eye05 = consts.tile([P128, 64], F32)  # 0.05 * I_8 flattened (m,n)
        nc.vector.memset(eye05, 0.0)
        nc.vector.memset(_ap(eye05, 0, [[9, 8]]), 0.05)

        # ades for all tiles: [128, (t, a)]
        ades_all = consts.tile([P128, T, AD], F32)
        nc.sync.dma_start(
            out=ades_all,
            in_=bass.AP(tensor=d_ades.tensor, offset=0,
                        ap=[[AD, P128], [P128 * AD, T], [1, AD]]),
        )

        # xT preloaded for all tiles; two dummy matmuls absorb the DMA waits on
        # PE's vector clock so every real matmul carries at most one sync wait
        # (walrus codegen limit on S3_LW).
        xaT_all = consts.tile([65, S], F32)
        nc.sync.dma_start(out=xaT_all, in_=d_xT)
        dummy_ps = psum1.tile([1, 1], F32, tag="dummy", bufs=1)
        dum1 = nc.tensor.matmul(dummy_ps, lhsT=qker[:, 0:1], rhs=qker[:, 0:1],
                                start=True, stop=True)
        dum2 = nc.tensor.matmul(dummy_ps, lhsT=xaT_all[:, 0:1], rhs=xaT_all[:, 0:1],
                                start=True, stop=True)
        add_dep_helper(dum2.ins, dum1.ins, sync=False, reason="pe presync order")

        # --- solver-wide buffers ---
        P_all = work.tile([P128, T * 64], F32)
        q_all = work.tile([P128, T * 8], F32)
        W_all = work.tile([P128, T * 128], F32)

        def apv(t, off, dims):
            return _ap(t, off, dims)

        # ---------------- per-tile prep ----------------
        for t in range(T):
            r0 = t * P128
            xa = dpool.tile([P128, XD], F32, tag="xa")
            nc.sync.dma_start(out=xa, in_=d_x[r0:r0 + P128, :])
            xaT = xaT_all[:, r0:r0 + P128]

            gh_ps = psum.tile([P128, 512], F32, tag="gh_ps")
            mm1 = nc.tensor.matmul(gh_ps, lhsT=xaT, rhs=qker[:, 0:512], start=True, stop=True)
            ha_ps = psum1.tile([P128, 8], F32, tag="ha_ps")
            mm2 = nc.tensor.matmul(ha_ps, lhsT=xaT, rhs=qker[:, 512:520], start=True, stop=True)
            add_dep_helper(mm1.ins, dum2.ins, sync=False, reason="pe presync order")
            add_dep_helper(mm2.ins, dum2.ins, sync=False, reason="pe presync order")
            gh = small.tile([P128, 512], F32, tag="gh")
            nc.scalar.copy(gh, gh_ps)
            haff = small.tile([P128, 8], F32, tag="haff")
            nc.scalar.copy(haff, ha_ps)

            # --- Ax ---
            At = dpool.tile([P128, XD * XD], F32, tag="A")
            nc.sync.dma_start(out=At, in_=d_A[r0:r0 + P128, :])
            # product laid out (j, i) with the contraction axis j OUTERMOST so the
            # DMA tree-add level below is a fully contiguous SBUF->SBUF transfer
            prodA = work.tile([P128, XD * XD], F32, tag="prodA", bufs=3)
            eng = nc.gpsimd if (gpsimd_offload and t % 3 != 0) else nc.vector
            eng.tensor_tensor(
                out=apv(prodA, 0, [[64, 64], [1, 64]]),
                in0=apv(At, 0, [[1, 64], [64, 64]]),
                in1=apv(xa, 0, [[1, 64], [0, 64]]),
                op=OP.mult,
            )
            Axv = small.tile([P128, XD], F32, tag="Ax")
            nc.vector.tensor_reduce(
                out=Axv, in_=apv(prodA, 0, [[1, 64], [64, 64]]), axis=AX.X, op=OP.add
            )

            # --- W = gh @ B  (per-sample, contraction over i) ---
            Bt = dpool.tile([P128, XD * AD], F32, tag="B")
            nc.sync.dma_start(out=Bt, in_=d_B[r0:r0 + P128, :])
            engW = nc.gpsimd if gpsimd_offload else nc.vector
            for half in (0, 1):
                prodW = work.tile([P128, 4 * AD * XD], F32, tag="prodW", bufs=3)
                engW.tensor_tensor(
                    out=apv(prodW, 0, [[64, 64], [16, 4], [1, 16]]),
                    in0=apv(gh, 256 * half, [[1, 64], [64, 4], [0, 16]]),
                    in1=apv(Bt, 0, [[16, 64], [0, 4], [1, 16]]),
                    op=OP.mult,
                )
                Wt = apv(W_all, 128 * t + 64 * half, [[16, 4], [1, 16]])
                nc.vector.tensor_reduce(
                    out=Wt,
                    in_=apv(prodW, 0, [[16, 4], [1, 16], [64, 64]]),
                    axis=AX.X, op=OP.add,
                )

            if debug and t == 0:
                nc.sync.dma_start(out=dbg["gh"], in_=gh)
                nc.sync.dma_start(out=dbg["Ax"], in_=Axv)
                nc.sync.dma_start(out=dbg["haffv"], in_=haff)

            # --- dots: ghx, ghAx ---
            prodD = work.tile([P128, 512], F32, tag="prodD")
            ghx = small.tile([P128, 8], F32, tag="ghx")
            nc.vector.tensor_tensor(
                out=prodD, in0=gh,
                in1=apv(xa, 0, [[0, 8], [1, 64]]), op=OP.mult)
            nc.vector.tensor_reduce(
                out=ghx, in_=prodD.rearrange("p (m i) -> p m i", m=8), axis=AX.X, op=OP.add)
            prodE = work.tile([P128, 512], F32, tag="prodE")
            ghAx = small.tile([P128, 8], F32, tag="ghAx")
            nc.vector.tensor_tensor(
                out=prodE, in0=gh,
                in1=apv(Axv, 0, [[0, 8], [1, 64]]), op=OP.mult)
            nc.vector.tensor_reduce(
                out=ghAx, in_=prodE.rearrange("p (m i) -> p m i", m=8), axis=AX.X, op=OP.add)

            # --- h = 0.5*ghx + haff ;  Wad ; q1 = -ghAx - h - Wad ---
            hv = small.tile([P128, 8], F32, tag="hv")
            nc.vector.scalar_tensor_tensor(
                out=hv, in0=ghx, scalar=0.5, in1=haff, op0=OP.mult, op1=OP.add)
            Wfull = apv(W_all, 128 * t, [[16, 8], [1, 16]])
            prodw2 = work.tile([P128, 128], F32, tag="prodw2")
            nc.vector.tensor_tensor(
                out=prodw2, in0=Wfull,
                in1=apv(ades_all, AD * t, [[0, 8], [1, 16]]), op=OP.mult)
            Wad = small.tile([P128, 8], F32, tag="Wad")
            nc.vector.tensor_reduce(
                out=Wad, in_=prodw2.rearrange("p (m a) -> p m a", m=8), axis=AX.X, op=OP.add)
            s1 = small.tile([P128, 8], F32, tag="s1")
            nc.vector.tensor_tensor(out=s1, in0=ghAx, in1=hv, op=OP.add)
            qt = apv(q_all, 8 * t, [[1, 8]])
            nc.vector.scalar_tensor_tensor(
                out=qt, in0=s1, scalar=-1.0, in1=Wad, op0=OP.mult, op1=OP.subtract)

            # --- P = 0.5 * W W^T + 0.05 I ---
            prodP = work.tile([P128, 1024], F32, tag="prodP")
            nc.vector.tensor_tensor(
                out=prodP,
                in0=apv(W_all, 128 * t, [[16, 8], [0, 8], [1, 16]]),
                in1=apv(W_all, 128 * t, [[0, 8], [16, 8], [1, 16]]),
                op=OP.mult,
            )
            Pww = work.tile([P128, 64], F32, tag="Pww")
            nc.vector.tensor_reduce(
                out=Pww, in_=prodP.rearrange("p (m n a) -> p m n a", m=8, n=8),
                axis=AX.X, op=OP.add)
            Pt = apv(P_all, 64 * t, [[8, 8], [1, 8]])
            nc.vector.scalar_tensor_tensor(
                out=Pt, in0=Pww, scalar=0.5, in1=eye05, op0=OP.mult, op1=OP.add)

        # ---------------- PDAS solver ----------------
        # Split into two independent halves so the front half's solve can
        # overlap the back tiles' prep (deps only reach P_all/q_all columns
        # of its own tiles).
        lam = work.tile([P128, T * 8], F32)
        mu = work.tile([P128, T * 8], F32)
        Dm = work.tile([P128, T * 8], F32)
        Em = work.tile([P128, T * 8], F32)
        sv = work.tile([P128, T * 8], F32)
        z = work.tile([P128, T * 8], F32)
        rd = work.tile([P128, T * 8], F32)
        Pm = work.tile([P128, T * 64], F32)
        tmp1 = work.tile([P128, T * 64], F32)
        tmpv = work.tile([P128, T * 8], F32)
        tmpw = work.tile([P128, T], F32)

        def run_solver(g0, G):
            qo = 8 * g0
            po = 64 * g0
            vq = [[1, 8 * G]]
            nc.vector.memset(apv(lam, qo, vq), 0.0)
            nc.vector.tensor_scalar(out=apv(mu, qo, vq), in0=apv(q_all, qo, vq),
                                    scalar1=-1.0, scalar2=None, op0=OP.mult)
            for it in range(PDAS_ITERS):
                nc.vector.tensor_tensor(out=apv(Dm, qo, vq), in0=apv(lam, qo, vq),
                                        in1=apv(mu, qo, vq), op=OP.is_gt)
                nc.vector.tensor_scalar(out=apv(Em, qo, vq), in0=apv(Dm, qo, vq),
                                        scalar1=-1.0, scalar2=1.0, op0=OP.mult, op1=OP.add)
                # Pm = P * (D x D) + diag(E)   (Pm/tmp1 scratch at offset 0)
                nc.vector.tensor_tensor(
                    out=apv(tmp1, 0, [[64, G], [8, 8], [1, 8]]),
                    in0=apv(P_all, po, [[64, G], [8, 8], [1, 8]]),
                    in1=apv(Dm, qo, [[8, G], [1, 8], [0, 8]]), op=OP.mult)
                nc.vector.tensor_tensor(
                    out=apv(Pm, 0, [[64, G], [8, 8], [1, 8]]),
                    in0=apv(tmp1, 0, [[64, G], [8, 8], [1, 8]]),
                    in1=apv(Dm, qo, [[8, G], [0, 8], [1, 8]]), op=OP.mult)
                diag = apv(Pm, 0, [[64, G], [9, 8]])
                nc.vector.tensor_tensor(out=diag, in0=diag,
                                        in1=apv(Em, qo, [[8, G], [1, 8]]), op=OP.add)
                nc.vector.tensor_tensor(out=apv(z, qo, vq), in0=apv(q_all, qo, vq),
                                        in1=apv(Dm, qo, vq), op=OP.mult)
                # masked LDL^T factorization (in place in Pm scratch)
                for k in range(8):
                    nc.vector.reciprocal(out=apv(rd, qo + k, [[8, G]]),
                                         in_=apv(Pm, 9 * k, [[64, G]]))
                    if k < 7:
                        r = 7 - k
                        col = apv(Pm, 8 * (k + 1) + k, [[64, G], [8, r]])
                        nc.vector.tensor_tensor(
                            out=col, in0=col,
                            in1=apv(rd, qo + k, [[8, G], [0, r]]), op=OP.mult)
                        tr = apv(Pm, 9 * (k + 1), [[64, G], [8, r], [1, r]])
                        ou = apv(tmp1, 0, [[64, G], [8, r], [1, r]])
                        nc.vector.tensor_tensor(
                            out=ou,
                            in0=apv(Pm, 8 * (k + 1) + k, [[64, G], [8, r], [0, r]]),
                            in1=apv(Pm, 9 * k + 1, [[64, G], [0, r], [1, r]]),
                            op=OP.mult)
                        nc.vector.tensor_tensor(out=tr, in0=tr, in1=ou, op=OP.subtract)
                # forward substitution
                for k in range(7):
                    r = 7 - k
                    tv = apv(tmpv, 0, [[8, G], [1, r]])
                    nc.vector.tensor_tensor(
                        out=tv,
                        in0=apv(Pm, 8 * (k + 1) + k, [[64, G], [8, r]]),
                        in1=apv(z, qo + k, [[8, G], [0, r]]), op=OP.mult)
                    zr = apv(z, qo + k + 1, [[8, G], [1, r]])
                    nc.vector.tensor_tensor(out=zr, in0=zr, in1=tv, op=OP.subtract)
                nc.vector.tensor_tensor(out=apv(z, qo, vq), in0=apv(z, qo, vq),
                                        in1=apv(rd, qo, vq), op=OP.mult)
                # backward substitution
                for k in range(6, -1, -1):
                    r = 7 - k
                    tv = apv(tmpv, 0, [[8, G], [1, r]])
                    nc.vector.tensor_tensor(
                        out=tv,
                        in0=apv(Pm, 8 * (k + 1) + k, [[64, G], [8, r]]),
                        in1=apv(z, qo + k + 1, [[8, G], [1, r]]), op=OP.mult)
                    red = apv(tmpw, 0, [[1, G]])
                    nc.vector.tensor_reduce(
                        out=red, in_=apv(tmpv, 0, [[8, G], [1, r]]), axis=AX.X, op=OP.add)
                    zk = apv(z, qo + k, [[8, G]])
                    nc.vector.tensor_tensor(out=zk, in0=zk, in1=red, op=OP.subtract)
                nc.vector.tensor_tensor(out=apv(lam, qo, vq), in0=apv(z, qo, vq),
                                        in1=apv(Dm, qo, vq), op=OP.mult)
                if it < PDAS_ITERS - 1:
                    nc.vector.tensor_tensor(
                        out=apv(tmp1, 0, [[64, G], [8, 8], [1, 8]]),
                        in0=apv(P_all, po, [[64, G], [8, 8], [1, 8]]),
                        in1=apv(lam, qo, [[8, G], [0, 8], [1, 8]]), op=OP.mult)
                    nc.vector.tensor_reduce(
                        out=apv(mu, qo, [[8, G], [1, 8]]),
                        in_=apv(tmp1, 0, [[64, G], [8, 8], [1, 8]]),
                        axis=AX.X, op=OP.add)
                    nc.vector.tensor_tensor(out=apv(mu, qo, vq), in0=apv(mu, qo, vq),
                                            in1=apv(q_all, qo, vq), op=OP.subtract)
            nc.vector.tensor_scalar(out=apv(lam, qo, vq), in0=apv(lam, qo, vq),
                                    scalar1=0.0, scalar2=None, op0=OP.max)

        run_solver(0, T)
        if debug:
            nc.sync.dma_start(out=dbg["W"], in_=W_all)
            nc.sync.dma_start(out=dbg["Pmat"], in_=P_all)
            nc.sync.dma_start(out=dbg["qv"], in_=q_all)
            nc.sync.dma_start(out=dbg["lamv"], in_=lam)

        # ---------------- recovery: a = a_des + 0.5 W^T lam ----------------
        for t in range(T):
            prodR = work.tile([P128, 128], F32, tag="prodR")
            # write product in (a, m)-physical order: out dims (m, a) strides [1, 8]
            nc.vector.tensor_tensor(
                out=apv(prodR, 0, [[1, 8], [8, 16]]),
                in0=apv(W_all, 128 * t, [[16, 8], [1, 16]]),
                in1=apv(lam, 8 * t, [[1, 8], [0, 16]]),
                op=OP.mult)
            sR = small.tile([P128, 16], F32, tag="sR")
            nc.vector.tensor_reduce(
                out=sR, in_=apv(prodR, 0, [[8, 16], [1, 8]]), axis=AX.X, op=OP.add)
            aout = small.tile([P128, 16], F32, tag="aout")
            nc.vector.scalar_tensor_tensor(
                out=aout, in0=sR, scalar=0.5,
                in1=apv(ades_all, AD * t, [[1, 16]]), op0=OP.mult, op1=OP.add)
            nc.sync.dma_start(out=d_out[t * P128:(t + 1) * P128, :], in_=aout)

    nc.compile()
    return nc


def _prep_inputs(a_des, x, A, B, Qc, cc, dc, S):
    qk = _qker_const(np.asarray(Qc, np.float32), np.asarray(cc, np.float32),
                     np.asarray(dc, np.float32))
    n = a_des.shape[0] // S
    maps = []
    for c in range(n):
        sl = slice(c * S, (c + 1) * S)
        maps.append({
            "x": np.ascontiguousarray(np.asarray(x, np.float32)[sl]),
            "xT": np.ascontiguousarray(
                np.concatenate([np.asarray(x, np.float32)[sl].T,
                                np.ones((1, S), np.float32)], axis=0)),
            "a_des": np.ascontiguousarray(np.asarray(a_des, np.float32)[sl]),
            "A": np.ascontiguousarray(np.asarray(A, np.float32)[sl].reshape(S, -1)),
            "B": np.ascontiguousarray(np.asarray(B, np.float32)[sl].reshape(S, -1)),
            "qker": qk,
        })
    return maps


def kernel(a_des, x, A, B, Qc, cc, dc):
    global _last_result
    from concourse.bass_utils import run_bass_kernel_spmd

    a_des = np.asarray(a_des, np.float32)
    S = a_des.shape[0] // NCORES
    nc = build_program(S)
    in_maps = _prep_inputs(a_des, x, A, B, Qc, cc, dc, S)
    t0 = time.time()
    res = run_bass_kernel_spmd(nc, in_maps, core_ids=list(range(NCORES)))
    _exec_wall[0] = time.time() - t0
    _last_result = res
    out = np.concatenate([r["a_safe"] for r in res.results], axis=0)
    return out.astype(np.float32)



# revision 7
# speedup vs baseline: 1.6055x; 1.6055x over previous
"""Batched CBF-QP safety filter on 8 Trainium2 NeuronCores.

v2: PE-centric. All per-sample contractions run as small per-sample matmuls
on the tensor engine using host-transposed layouts:
  - ghT[i,(m,s)] from shared-weight matmuls (qker), incl. cc affine via ones row.
  - v = A x via per-sample stationary Ajp (A^T per sample, j-on-partitions,
    augmented 65th row/col so vT row 64 develops the constant for vx_aug).
  - W^T[a,m] per sample via stationary Bi (B with i-on-partitions).
  - q = -(ghAx + h + Wad) accumulated in PSUM via 3 per-sample matmuls.
  - P = 0.5 W W^T + 0.05I via gpsimd outer product + DVE reduce from W_s
    (W_s obtained by 8 PE transposes of WT).
  - 8-dim NNQP solved by 5 primal-dual active set iterations on DVE (SIMD
    over samples), recovery a = a_des + 0.5 W^T lam on DVE.
Batch 16384 -> 8 cores x 2048 samples; 16 tiles of 128 samples per core.
"""

import os
import time
from contextlib import ExitStack

import numpy as np

import concourse.bacc as bacc
import concourse.bass as bass
import concourse.mybir as mybir
import concourse.tile as tile
from concourse.tile_rust import add_dep_helper

F32 = mybir.dt.float32
OP = mybir.AluOpType
AX = mybir.AxisListType

BATCH = 16384
XD = 64
AD = 16
NC = 8
PEN = 10.0
DELTA = 1.0
NCORES = 8
P128 = 128
PDAS_ITERS = 5

_last_result = None
_exec_wall = [None]


def _qker_const(Qc: np.ndarray, cc: np.ndarray, dc: np.ndarray) -> np.ndarray:
    """[65, 520] fp32: gh columns (512, (m,i) major) + affine-h columns (8)."""
    k = np.zeros((65, 520), np.float32)
    k[:64, :512] = -np.transpose(Qc, (2, 0, 1)).reshape(64, 512)
    k[64, :512] = -cc.reshape(512)
    k[:64, 512:520] = -0.5 * cc.T
    k[64, 512:520] = dc
    return k


def _ap(base: bass.AP, off_elems: int, dims):
    """Custom free-dim view of an SBUF/PSUM tile AP (keeps partition dim)."""
    return bass.AP(
        tensor=base.tensor,
        offset=base.offset + off_elems,
        ap=[list(base.ap[0])] + [list(d) for d in dims],
    )


def build_program(S: int, gpsimd_offload: bool = True, debug: bool = False):
    """Build the per-core Bass program for S samples (S % 128 == 0)."""
    T = S // P128
    nc = bacc.Bacc("TRN2", target_bir_lowering=False)

    d_xaT = nc.dram_tensor("xaT", [65, S], F32, kind="ExternalInput").ap()
    d_adesT = nc.dram_tensor("adesT", [AD, S], F32, kind="ExternalInput").ap()
    d_ades = nc.dram_tensor("a_des", [S, AD], F32, kind="ExternalInput").ap()
    d_Ajp = nc.dram_tensor("Ajp", [T * 64, 64 * P128], F32, kind="ExternalInput").ap()
    d_Bi = nc.dram_tensor("Bi", [T * 64, AD * P128], F32, kind="ExternalInput").ap()
    d_qk = nc.dram_tensor("qker", [65, 520], F32, kind="ExternalInput").ap()
    d_id = nc.dram_tensor("ident", [16, 16], F32, kind="ExternalInput").ap()
    d_out = nc.dram_tensor("a_safe", [S, AD], F32, kind="ExternalOutput").ap()

    with tile.TileContext(nc) as tc, ExitStack() as ctx:
        consts = ctx.enter_context(tc.tile_pool(name="consts", bufs=1))
        dpool = ctx.enter_context(tc.tile_pool(name="dma", bufs=2))
        work = ctx.enter_context(tc.tile_pool(name="work", bufs=1))
        small = ctx.enter_context(tc.tile_pool(name="small", bufs=2))
        psA = ctx.enter_context(tc.tile_pool(name="psA", bufs=1, space="PSUM"))
        psB = ctx.enter_context(tc.tile_pool(name="psB", bufs=2, space="PSUM"))
        psC = ctx.enter_context(tc.tile_pool(name="psC", bufs=1, space="PSUM"))
        psD = ctx.enter_context(tc.tile_pool(name="psD", bufs=2, space="PSUM"))

        # --- constants ---
        qker = consts.tile([65, 520], F32)
        nc.sync.dma_start(out=qker, in_=d_qk)
        ident = consts.tile([16, 16], F32)
        nc.sync.dma_start(out=ident, in_=d_id)
        eye05 = consts.tile([P128, 64], F32)  # 0.05 * I_8 flattened (m,n)
        nc.vector.memset(eye05, 0.0)
        nc.vector.memset(_ap(eye05, 0, [[9, 8]]), 0.05)

        ades_all = consts.tile([P128, T, AD], F32)
        nc.sync.dma_start(
            out=ades_all,
            in_=bass.AP(tensor=d_ades.tensor, offset=0,
                        ap=[[AD, P128], [P128 * AD, T], [1, AD]]),
        )
        xaT_all = consts.tile([65, S], F32)
        nc.sync.dma_start(out=xaT_all, in_=d_xaT)
        adesT_all = consts.tile([AD, S], F32)
        nc.sync.dma_start(out=adesT_all, in_=d_adesT)

        # --- solver-wide buffers ---
        P_all = work.tile([P128, T * 64], F32)
        q_all = work.tile([P128, T * 8], F32)
        W_all = work.tile([P128, T * 128], F32)

        # ---------------- per-tile prep ----------------
        for t in range(T):
            r0 = t * P128
            Ajp = dpool.tile([64, 64 * P128], F32, tag="Ajp")
            nc.sync.dma_start(out=Ajp, in_=d_Ajp[t * 64:(t + 1) * 64, :])
            Bi = dpool.tile([64, AD * P128], F32, tag="Bi")
            nc.sync.dma_start(out=Bi, in_=d_Bi[t * 64:(t + 1) * 64, :])

            bankA = psA.tile([64, 1024], F32, tag="ghT")       # ghT[i, (m*128+s)]
            bankB = psB.tile([P128, 512], F32, tag="bankB")    # vT|haffT|q_acc|W_s
            bankC = psC.tile([16, 1024], F32, tag="WT")        # WT[a, (s*8+m)]
            q_s = psD.tile([P128, 8], F32, tag="q_s")

            # ghT: 8 shared-weight matmuls -> [i, s] per m
            for m in range(8):
                nc.tensor.matmul(bankA[:, 128 * m:128 * (m + 1)],
                                 lhsT=qker[:, 64 * m:64 * m + 64],
                                 rhs=xaT_all[:, r0:r0 + P128],
                                 start=True, stop=True)
            # haffT [8, s] at bankB cols 128:256
            nc.tensor.matmul(bankB[0:8, 128:256],
                             lhsT=qker[:, 512:520],
                             rhs=xaT_all[:, r0:r0 + P128],
                             start=True, stop=True)

            # ghT to SBUF [64, (m*128+s)] (same partitions, 2 ACT copies)
            ghT_aug = small.tile([64, 1024], F32, tag="ghT_aug")
            nc.scalar.copy(ghT_aug[:, 0:512], bankA[:, 0:512])
            nc.scalar.copy(ghT_aug[:, 512:1024], bankA[:, 512:1024])

            # v per-sample matmuls: vT [64, s] in bankB cols 0:128
            for s in range(P128):
                nc.tensor.matmul(bankB[0:64, s:s + 1],
                                 lhsT=Ajp[:, 64 * s:64 * (s + 1)],
                                 rhs=xaT_all[0:64, r0 + s:r0 + s + 1],
                                 start=True, stop=True)

            # W per-sample matmuls: WT[a, (s*8+m)] -> bankC
            for s in range(P128):
                nc.tensor.matmul(bankC[:, 8 * s:8 * s + 8],
                                 lhsT=Bi[:, AD * s:AD * (s + 1)],
                                 rhs=_ap(ghT_aug, s, [[128, 8]]),
                                 start=True, stop=True)

            # WT to SBUF (same partitions)
            WT_sb = small.tile([16, 1024], F32, tag="WT_sb")
            nc.scalar.copy(WT_sb, bankC)

            # vx = 0.5 * xT + vT
            vx_aug = small.tile([64, P128], F32, tag="vx")
            nc.vector.scalar_tensor_tensor(
                out=vx_aug, in0=xaT_all[0:64, r0:r0 + P128], scalar=0.5,
                in1=bankB[0:64, 0:128], op0=OP.mult, op1=OP.add)

            # q accumulation: q_acc[m, s] = ghAx + 0.5*ghx + Wad at bankB cols 256:384
            for s in range(P128):
                nc.tensor.matmul(bankB[0:8, 256 + s:257 + s],
                                 lhsT=_ap(ghT_aug, s, [[128, 8]]),
                                 rhs=vx_aug[:, s:s + 1],
                                 start=True, stop=False, skip_group_check=True)
                nc.tensor.matmul(bankB[0:8, 256 + s:257 + s],
                                 lhsT=WT_sb[:, 8 * s:8 * s + 8],
                                 rhs=adesT_all[:, r0 + s:r0 + s + 1],
                                 start=False, stop=True, skip_group_check=True)

            # q2 = q_acc + haffT -> SBUF -> transpose -> q_all (negated)
            haff_sb = small.tile([8, P128], F32, tag="haff_sb")
            nc.scalar.copy(haff_sb, bankB[0:8, 128:256])
            q_sb = small.tile([8, P128], F32, tag="q_sb")
            nc.vector.tensor_tensor(out=q_sb, in0=bankB[0:8, 256:384],
                                    in1=haff_sb, op=OP.add)
            nc.tensor.transpose(q_s, in_=q_sb, identity=ident[0:8, 0:8])
            nc.vector.tensor_scalar(out=_ap(q_all, 8 * t, [[1, 8]]), in0=q_s,
                                    scalar1=-1.0, scalar2=None, op0=OP.mult)

            # W_s [s, (m*16+a)] via 8 PE transposes of WT -> bankB cols 384:512
            Wsb = bankB[:, 384:512]
            for m in range(8):
                nc.tensor.transpose(
                    _ap(Wsb, 16 * m, [[1, 16]]),
                    in_=_ap(WT_sb, m, [[8, 128]]),
                    identity=ident)
            nc.scalar.copy(_ap(W_all, 128 * t, [[1, 128]]), Wsb)

            # P = 0.5 W W^T + 0.05I on gpsimd(mult) + DVE(reduce, stt)
            prodP = work.tile([P128, 1024], F32, tag="prodP", bufs=2)
            nc.gpsimd.tensor_tensor(
                out=_ap(prodP, 0, [[128, 8], [16, 8], [1, 16]]),
                in0=_ap(W_all, 128 * t, [[16, 8], [0, 8], [1, 16]]),
                in1=_ap(W_all, 128 * t, [[0, 8], [16, 8], [1, 16]]),
                op=OP.mult)
            Pww = work.tile([P128, 64], F32, tag="Pww", bufs=2)
            nc.vector.tensor_reduce(
                out=Pww, in_=_ap(prodP, 0, [[128, 8], [16, 8], [1, 16]]),
                axis=AX.X, op=OP.add)
            nc.vector.scalar_tensor_tensor(
                out=_ap(P_all, 64 * t, [[8, 8], [1, 8]]), in0=Pww, scalar=0.5,
                in1=eye05, op0=OP.mult, op1=OP.add)

        # ---------------- PDAS solver ----------------
        lam = work.tile([P128, T * 8], F32)
        mu = work.tile([P128, T * 8], F32)
        Dm = work.tile([P128, T * 8], F32)
        Em = work.tile([P128, T * 8], F32)
        sv = work.tile([P128, T * 8], F32)
        z = work.tile([P128, T * 8], F32)
        rd = work.tile([P128, T * 8], F32)
        Pm = work.tile([P128, T * 64], F32)
        tmp1 = work.tile([P128, T * 64], F32)
        tmpv = work.tile([P128, T * 8], F32)
        tmpw = work.tile([P128, T], F32)

        def apv(t_, off, dims):
            return _ap(t_, off, dims)

        def run_solver(g0, G):
            qo = 8 * g0
            po = 64 * g0
            vq = [[1, 8 * G]]
            nc.vector.memset(apv(lam, qo, vq), 0.0)
            nc.vector.tensor_scalar(out=apv(mu, qo, vq), in0=apv(q_all, qo, vq),
                                    scalar1=-1.0, scalar2=None, op0=OP.mult)
            for it in range(PDAS_ITERS):
                nc.vector.tensor_tensor(out=apv(Dm, qo, vq), in0=apv(lam, qo, vq),
                                        in1=apv(mu, qo, vq), op=OP.is_gt)
                nc.vector.tensor_scalar(out=apv(Em, qo, vq), in0=apv(Dm, qo, vq),
                                        scalar1=-1.0, scalar2=1.0, op0=OP.mult, op1=OP.add)
                nc.vector.tensor_tensor(
                    out=apv(tmp1, 0, [[64, G], [8, 8], [1, 8]]),
                    in0=apv(P_all, po, [[64, G], [8, 8], [1, 8]]),
                    in1=apv(Dm, qo, [[8, G], [1, 8], [0, 8]]), op=OP.mult)
                nc.vector.tensor_tensor(
                    out=apv(Pm, 0, [[64, G], [8, 8], [1, 8]]),
                    in0=apv(tmp1, 0, [[64, G], [8, 8], [1, 8]]),
                    in1=apv(Dm, qo, [[8, G], [0, 8], [1, 8]]), op=OP.mult)
                diag = apv(Pm, 0, [[64, G], [9, 8]])
                nc.vector.tensor_tensor(out=diag, in0=diag,
                                        in1=apv(Em, qo, [[8, G], [1, 8]]), op=OP.add)
                nc.vector.tensor_tensor(out=apv(z, qo, vq), in0=apv(q_all, qo, vq),
                                        in1=apv(Dm, qo, vq), op=OP.mult)
                for k in range(8):
                    nc.vector.reciprocal(out=apv(rd, qo + k, [[8, G]]),
                                         in_=apv(Pm, 9 * k, [[64, G]]))
                    if k < 7:
                        r = 7 - k
                        col = apv(Pm, 8 * (k + 1) + k, [[64, G], [8, r]])
                        nc.vector.tensor_tensor(
                            out=col, in0=col,
                            in1=apv(rd, qo + k, [[8, G], [0, r]]), op=OP.mult)
                        tr = apv(Pm, 9 * (k + 1), [[64, G], [8, r], [1, r]])
                        ou = apv(tmp1, 0, [[64, G], [8, r], [1, r]])
                        nc.vector.tensor_tensor(
                            out=ou,
                            in0=apv(Pm, 8 * (k + 1) + k, [[64, G], [8, r], [0, r]]),
                            in1=apv(Pm, 9 * k + 1, [[64, G], [0, r], [1, r]]),
                            op=OP.mult)
                        nc.vector.tensor_tensor(out=tr, in0=tr, in1=ou, op=OP.subtract)
                for k in range(7):
                    r = 7 - k
                    tv = apv(tmpv, 0, [[8, G], [1, r]])
                    nc.vector.tensor_tensor(
                        out=tv,
                        in0=apv(Pm, 8 * (k + 1) + k, [[64, G], [8, r]]),
                        in1=apv(z, qo + k, [[8, G], [0, r]]), op=OP.mult)
                    zr = apv(z, qo + k + 1, [[8, G], [1, r]])
                    nc.vector.tensor_tensor(out=zr, in0=zr, in1=tv, op=OP.subtract)
                nc.vector.tensor_tensor(out=apv(z, qo, vq), in0=apv(z, qo, vq),
                                        in1=apv(rd, qo, vq), op=OP.mult)
                for k in range(6, -1, -1):
                    r = 7 - k
                    tv = apv(tmpv, 0, [[8, G], [1, r]])
                    nc.vector.tensor_tensor(
                        out=tv,
                        in0=apv(Pm, 8 * (k + 1) + k, [[64, G], [8, r]]),
                        in1=apv(z, qo + k + 1, [[8, G], [1, r]]), op=OP.mult)
                    red = apv(tmpw, 0, [[1, G]])
                    nc.vector.tensor_reduce(
                        out=red, in_=apv(tmpv, 0, [[8, G], [1, r]]), axis=AX.X, op=OP.add)
                    zk = apv(z, qo + k, [[8, G]])
                    nc.vector.tensor_tensor(out=zk, in0=zk, in1=red, op=OP.subtract)
                nc.vector.tensor_tensor(out=apv(lam, qo, vq), in0=apv(z, qo, vq),
                                        in1=apv(Dm, qo, vq), op=OP.mult)
                if it < PDAS_ITERS - 1:
                    nc.vector.tensor_tensor(
                        out=apv(tmp1, 0, [[64, G], [8, 8], [1, 8]]),
                        in0=apv(P_all, po, [[64, G], [8, 8], [1, 8]]),
                        in1=apv(lam, qo, [[8, G], [0, 8], [1, 8]]), op=OP.mult)
                    nc.vector.tensor_reduce(
                        out=apv(mu, qo, [[8, G], [1, 8]]),
                        in_=apv(tmp1, 0, [[64, G], [8, 8], [1, 8]]),
                        axis=AX.X, op=OP.add)
                    nc.vector.tensor_tensor(out=apv(mu, qo, vq), in0=apv(mu, qo, vq),
                                            in1=apv(q_all, qo, vq), op=OP.subtract)
            nc.vector.tensor_scalar(out=apv(lam, qo, vq), in0=apv(lam, qo, vq),
                                    scalar1=0.0, scalar2=None, op0=OP.max)

        run_solver(0, T)

        # ---------------- recovery: a = a_des + 0.5 W^T lam ----------------
        for t in range(T):
            prodR = work.tile([P128, 128], F32, tag="prodR")
            nc.vector.tensor_tensor(
                out=_ap(prodR, 0, [[1, 8], [8, 16]]),
                in0=_ap(W_all, 128 * t, [[16, 8], [1, 16]]),
                in1=_ap(lam, 8 * t, [[1, 8], [0, 16]]),
                op=OP.mult)
            sR = small.tile([P128, 16], F32, tag="sR")
            nc.vector.tensor_reduce(
                out=sR, in_=_ap(prodR, 0, [[8, 16], [1, 8]]), axis=AX.X, op=OP.add)
            aout = small.tile([P128, 16], F32, tag="aout")
            nc.vector.scalar_tensor_tensor(
                out=aout, in0=sR, scalar=0.5,
                in1=_ap(ades_all, AD * t, [[1, 16]]), op0=OP.mult, op1=OP.add)
            nc.sync.dma_start(out=d_out[t * P128:(t + 1) * P128, :], in_=aout)

    nc.compile()
    return nc


def _prep_inputs(a_des, x, A, B, Qc, cc, dc, S):
    qk = _qker_const(np.asarray(Qc, np.float32), np.asarray(cc, np.float32),
                     np.asarray(dc, np.float32))
    ident = np.eye(16, dtype=np.float32)
    T = S // P128
    n = a_des.shape[0] // S
    a_des = np.asarray(a_des, np.float32)
    x = np.asarray(x, np.float32)
    A = np.asarray(A, np.float32)
    B = np.asarray(B, np.float32)
    maps = []
    for c in range(n):
        sl = slice(c * S, (c + 1) * S)
        xc = x[sl]
        Ac = A[sl].reshape(T, P128, 64, 64)
        Bc = B[sl].reshape(T, P128, 64, AD)
        Ajp = np.ascontiguousarray(Ac.transpose(0, 3, 1, 2))
        Bi = np.ascontiguousarray(Bc.transpose(0, 2, 1, 3))
        maps.append({
            "xaT": np.ascontiguousarray(
                np.concatenate([xc.T, np.ones((1, S), np.float32)], axis=0)),
            "adesT": np.ascontiguousarray(a_des[sl].T),
            "a_des": np.ascontiguousarray(a_des[sl]),
            "Ajp": Ajp.reshape(T * 64, 64 * P128),
            "Bi": Bi.reshape(T * 64, AD * P128),
            "qker": qk,
            "ident": ident,
        })
    return maps


def kernel(a_des, x, A, B, Qc, cc, dc):
    global _last_result
    from concourse.bass_utils import run_bass_kernel_spmd

    a_des = np.asarray(a_des, np.float32)
    S = a_des.shape[0] // NCORES
    nc = build_program(S)
    in_maps = _prep_inputs(a_des, x, A, B, Qc, cc, dc, S)
    t0 = time.time()
    res = run_bass_kernel_spmd(nc, in_maps, core_ids=list(range(NCORES)))
    _exec_wall[0] = time.time() - t0
    _last_result = res
    out = np.concatenate([r["a_safe"] for r in res.results], axis=0)
    return out.astype(np.float32)
